# revision 12
# baseline (speedup 1.0000x reference)
"""Causal multi-head attention (B=4, H=16, S=2048, D=64) on 8 TRN2 NeuronCores.

Sharding: B*H = 64 (batch, head) pairs -> 8 per core, fully independent,
no collectives.

Per-core algorithm (per head):
  - Q, K of each PAIR of heads are DMA-cast (f32->bf16, SWDGE) into one
    [S, 128] DRAM scratch ([qA | qB] column-packed), then one DMA-xbar
    transpose per scratch yields [128, S] SBUF tiles: partitions 0:64 =
    head A's Q^T (d on partitions), 64:128 = head B's. No zero padding,
    K(contraction)=64 matmuls.
  - For each k-block kb (128 keys): S^T[kb] = matmul(lhsT=KT block
    [64,128], rhs=QT[:, q>=kb*128]) in 512-col chunks -> PSUM [128,512],
    exp(0.125 x) on ScalarE -> U^T[kb] bf16. Diagonal block masked by
    upper-triangular multiply on GpSimd (Pool). Head 0's exp chunks
    alternate onto a custom 2-instruction DVE exp (deg-2 poly ^64) to
    shorten the un-pipelined first head.
  - PV runs ONE HEAD BEHIND QK, interleaved per block in PE program
    order, so the PE never head-of-line blocks on exp: O[qb] = sum_kb
    U^T[kb].T @ [V[kb] | 1] in PSUM [128,65]; col 64 = softmax denom;
    normalize with per-partition reciprocal multiply (DVE), store f32.
"""

import numpy as np

import concourse.bass as bass
import concourse.tile as tile
from concourse import mybir
from concourse import dve_ops as dvo
from concourse import dve_spec as dsp
from concourse.bass_utils import run_bass_kernel_spmd
from concourse.dve_table_gen import dve_ver_for
from concourse.dve_uop import DveOpSpec
from concourse.masks import make_upper_triangular
from concourse.vector_clock import ScopedClock, VectorClock

F32 = mybir.dt.float32
BF16 = mybir.dt.bfloat16

import os

USE_DVE_EXP = os.environ.get("USE_DVE_EXP", "") == "1"
PAIR_PACK = os.environ.get("USE_PAIR_PACK", "") == "1"
GPSIMD_OPS = os.environ.get("NO_GPSIMD_OPS", "") != "1"

B, H, S, D = 4, 16, 2048, 64
N_CORES = 8
HEADS_PER_CORE = B * H // N_CORES  # 8
NB = S // 128  # 16 blocks of 128
SCALE = 1.0 / np.sqrt(np.float32(D))  # 0.125

# Degree-2 minimax fit of exp(s/512) on s in [-56, 56]; kernel computes
# (poly)^16 on DVE op1 and ^4 on op2 => exp(s/8) with ~2e-3 rel err.
_EXPC = None


def _fit_exp_coeffs():
    global _EXPC
    if _EXPC is not None:
        return _EXPC
    s = np.linspace(-56.0, 56.0, 8001)
    t = np.exp(s / 512.0)
    w = 1.0 / t
    coef = None
    for _ in range(50):
        A = np.stack([np.ones_like(s), s, s * s], axis=1)
        coef, *_ = np.linalg.lstsq(A * w[:, None], t * w, rcond=None)
        rel = np.abs((A @ coef - t) / t)
        w = w * (1.0 + 5.0 * rel / rel.max())
        w /= w.max()
    _EXPC = (float(coef[0]), float(coef[1]), float(coef[2]))
    return _EXPC


def _register_dve_exp():
    """Register the 2-op DVE exp chain in dve_ops' registries (new rows in
    the 5-bit opcode table; shas pinned from a fresh lower())."""
    if "ANT_EXP_P16" in dvo._SUB_OPCODE_FOR_NAME:
        return (
            next(o for o in dvo.OPS if o.name == "ANT_EXP_P16"),
            next(o for o in dvo.OPS if o.name == "ANT_POW4"),
        )
    p = (dsp.Src0 * dsp.C0 + dsp.C1) * dsp.Src0 + dsp.C2
    body1 = dsp.sq(dsp.sq(dsp.sq(dsp.sq(p))))
    op1 = dvo.DveOp(
        "ANT_EXP_P16",
        dsp.Spec(
            body=body1,
            reference=lambda in0, in1, c0, c1, c2: (
                ((in0 * c0 + c1) * in0 + c2) ** 16.0
            ),
        ),
        subdim=False,
        uops_sha={},
    )
    op2 = dvo.DveOp(
        "ANT_POW4",
        dsp.Spec(
            body=dsp.sq(dsp.sq(dsp.Src0)),
            reference=lambda in0, in1, c0, c1, c2: in0**4.0,
        ),
        subdim=False,
        uops_sha={},
    )
    for op in (op1, op2):
        dvo.OPS.append(op)
        dvo.CUSTOM_DVE_SPECS[op.name] = op.spec
        dvo._SUB_OPCODE_FOR_NAME[op.name] = (
            max(dvo._SUB_OPCODE_FOR_NAME.values()) + 1
        )
        for ver in ("v3", "v4"):
            uops = dsp.lower(op.spec, ver=ver)
            sha = DveOpSpec(
                name=op.name,
                opcode=dvo.get_dve_sub_opcode(op.name),
                uops=uops,
                rd1_en=False,
            ).sha(ver)
            op.uops_sha[ver] = sha
    return op1, op2


def _patch_tile_drain():
    """This walrus build rejects >1 sem wait on the kernel-tail Drain
    instruction ("Too many sync wait commands"). Spread the waits across
    single-wait NOPs on the sync engine instead."""
    if getattr(tile.TileContext, "_drain_patched", False):
        return

    def _drain_and_barrier(self, tick_clock, wait_clock):
        gc = tick_clock.global_clock
        n = len(gc)
        for i in range(n):
            if gc[i] > 0:
                vc = VectorClock([gc[j] if j == i else 0 for j in range(n)])
                nop_inst = self.nc.sync.nop(nofuse=True, hint=f"drainwait{i}")
                wait_clock.add_sem_waits(nop_inst.ins, ScopedClock({None: vc}))
        self.nc.sync.drain()
        self.nc.all_engine_barrier()
        popped = self.nc._tile_sem_poison_stack.pop()
        assert popped is self._sem_poison
        self.nc.clear_and_free_semaphores(list(self.sems.allocated().values()))
        self.nc.all_engine_barrier()

    tile.TileContext._drain_and_barrier = _drain_and_barrier
    tile.TileContext._drain_patched = True


_patch_tile_drain()


def _split_multi_waits(nc, limit=1):
    """This walrus build allows at most one sem wait per instruction.
    Move excess waits onto same-engine NOPs inserted just before."""
    ctr = [0]
    for func in nc.m.functions:
        for bb in func.blocks:
            insts = list(bb.instructions)
            out = []
            changed = False
            for inst in insts:
                si = inst.sync_info
                if si is not None and si.on_wait is not None and len(si.on_wait) > limit:
                    waits = list(si.on_wait)
                    extra, keep = waits[:-limit], waits[-limit:]
                    for w in extra:
                        ctr[0] += 1
                        nop = mybir.InstNoOp(
                            name=f"waitsplit-{ctr[0]}", ins=[], outs=[]
                        )
                        nop.engine = inst.engine
                        nop.sync_info = mybir.SyncInfo(on_wait=[w], on_update=[])
                        out.append(nop)
                    inst.sync_info = mybir.SyncInfo(
                        on_wait=keep, on_update=list(si.on_update or [])
                    )
                    changed = True
                out.append(inst)
            if changed:
                try:
                    bb.instructions[:] = out
                except Exception:
                    bb.instructions = out
    return nc


def build_nc(n_heads: int = HEADS_PER_CORE):
    op_exp, op_pow4 = _register_dve_exp()
    c0, c1, c2 = _fit_exp_coeffs()

    nc = bass.Bass("TRN2", target_bir_lowering=False)
    q_d = nc.dram_tensor("queries", [n_heads, S, D], F32, kind="ExternalInput")
    k_d = nc.dram_tensor("keys", [n_heads, S, D], F32, kind="ExternalInput")
    v_d = nc.dram_tensor("values", [n_heads, S, D], F32, kind="ExternalInput")
    o_d = nc.dram_tensor("out", [n_heads, S, D], F32, kind="ExternalOutput")

    n_pairs = (n_heads + 1) // 2
    o_r = o_d[:].rearrange("h (n p) d -> h p n d", p=128)

    with tile.TileContext(nc) as tc:
        with (
            tc.tile_pool(name="const", bufs=1) as constp,
            tc.tile_pool(name="scr", bufs=2, space="DRAM") as scrp,
            tc.tile_pool(name="vscr", bufs=3, space="DRAM") as vscrp,
            tc.tile_pool(name="tp", bufs=2) as tpp,
            tc.tile_pool(name="vpool", bufs=5) as vpp,
            tc.tile_pool(name="ut", bufs=3) as utp,
            tc.tile_pool(name="etmp", bufs=4) as etp,
            tc.tile_pool(name="oh", bufs=2) as ohp,
            tc.tile_pool(name="rz", bufs=4) as rzp,
            tc.tile_pool(name="ps_s", bufs=5, space="PSUM") as ps_s,
            tc.tile_pool(name="ps_o", bufs=2, space="PSUM") as ps_o,
        ):
            trimask = constp.tile([128, 128], BF16)
            make_upper_triangular(nc, trimask, val=1.0, diag=True)
            if not PAIR_PACK:
                zpad = constp.tile([128, 1024], BF16)
                nc.vector.memset(zpad, 0.0)

            scrs = {}
            xps = {}
            vps = {}

            def issue_cast(pair):
                if not PAIR_PACK:
                    # baseline-style: per-head [S,128] scratch, cols 64:128
                    # zeroed.
                    for h in range(2 * pair, min(2 * pair + 2, n_heads)):
                        scrq = scrp.tile([S, 128], BF16, tag=f"scrq{h % 2}")
                        scrk = scrp.tile([S, 128], BF16, tag=f"scrk{h % 2}")
                        nc.gpsimd.dma_start(out=scrq[:, D : 2 * D], in_=zpad)
                        nc.gpsimd.dma_start(out=scrk[:, D : 2 * D], in_=zpad)
                        nc.gpsimd.dma_start(out=scrq[:, 0:D], in_=q_d[h])
                        nc.gpsimd.dma_start(out=scrk[:, 0:D], in_=k_d[h])
                        scrs[h] = (scrq, scrk)
                    return
                # [S, 128] scratch = [q(2p) | q(2p+1)] columns; one SWDGE
                # cast DMA per tensor reading two contiguous heads.
                hi0 = 2 * pair
                nh = min(2, n_heads - hi0)
                scrq = scrp.tile([S, 128], BF16, tag="scrq")
                scrk = scrp.tile([S, 128], BF16, tag="scrk")
                for j in range(nh):
                    nc.gpsimd.dma_start(
                        out=scrq[:, 64 * j : 64 * j + 64], in_=q_d[hi0 + j]
                    )
                    nc.gpsimd.dma_start(
                        out=scrk[:, 64 * j : 64 * j + 64], in_=k_d[hi0 + j]
                    )
                scrs[pair] = (scrq, scrk)

            def issue_xpose(pair):
                if not PAIR_PACK:
                    for h in range(2 * pair, min(2 * pair + 2, n_heads)):
                        scrq, scrk = scrs.pop(h)
                        qt2 = tpp.tile([128, S], BF16, tag=f"qt2{h % 2}")
                        kt2 = tpp.tile([128, S], BF16, tag=f"kt2{h % 2}")
                        nc.sync.dma_start(out=qt2, in_=scrq[:, :], transpose=True)
                        nc.sync.dma_start(out=kt2, in_=scrk[:, :], transpose=True)
                        xps[h] = (qt2, kt2)
                    return
                scrq, scrk = scrs.pop(pair)
                qt2 = tpp.tile([128, S], BF16, tag="qt2")
                kt2 = tpp.tile([128, S], BF16, tag="kt2")
                nc.sync.dma_start(out=qt2, in_=scrq[:, :], transpose=True)
                nc.sync.dma_start(out=kt2, in_=scrk[:, :], transpose=True)
                xps[pair] = (qt2, kt2)

            def issue_v(h):
                # contiguous SWDGE cast to DRAM, then HWDGE load on sync.
                vscr = vscrp.tile([S, D], BF16, tag="vscr")
                nc.gpsimd.dma_start(out=vscr, in_=v_d[h])
                vp = vpp.tile([128, NB, D + 1], BF16, tag="vp")
                v_r = vscr[:].rearrange("(n p) d -> p n d", p=128)
                nc.sync.dma_start(out=vp[:, :, 0:D], in_=v_r)
                if GPSIMD_OPS:
                    nc.gpsimd.memset(vp[:, :, D : D + 1], 1.0)
                else:
                    nc.vector.memset(vp[:, :, D : D + 1], 1.0)
                vps[h] = vp

            # prologue lookahead
            issue_cast(0)
            if n_pairs > 1:
                issue_cast(1)
            issue_xpose(0)
            for h in range(min(3, n_heads)):
                issue_v(h)

            def pv_block(hp, qb, uts, vp, oh):
                po = ps_o.tile([128, D + 1], F32, tag="o")
                for kb2 in range(qb + 1):
                    nc.tensor.matmul(
                        po,
                        lhsT=uts[kb2][:, (qb - kb2) * 128 : (qb - kb2) * 128 + 128],
                        rhs=vp[:, kb2, :],
                        start=(kb2 == 0),
                        stop=(kb2 == qb),
                    )
                rz = rzp.tile([128, 1], F32, tag="rz")
                nc.vector.reciprocal(rz, po[:, D : D + 1])
                nc.vector.tensor_scalar_mul(oh[:, qb, :], po[:, 0:D], rz)

            prev = None  # (uts, vp, oh) of head h-1
            for h in range(n_heads + 1):
                cur = None
                if h < n_heads:
                    pair, par = divmod(h, 2)
                    if par == 0:
                        if pair + 2 < n_pairs:
                            issue_cast(pair + 2)
                        if pair + 1 < n_pairs:
                            issue_xpose(pair + 1)
                    if h + 3 < n_heads:
                        issue_v(h + 3)
                    if PAIR_PACK:
                        qt2, kt2 = xps[pair]
                        qt = qt2[64 * par : 64 * par + 64, :]
                        kt = kt2[64 * par : 64 * par + 64, :]
                    else:
                        qt, kt = xps[h]
                    vp = vps.pop(h)
                    oh = ohp.tile([128, NB, D], F32, tag="oh")
                    uts = []
                    cur = (uts, vp, oh)

                for kb in range(NB):
                    if h < n_heads:
                        uts, vp, oh = cur
                        qlo = kb * 128
                        L = S - qlo
                        ut = utp.tile([128, L], BF16, tag=f"ut{kb}")
                        uts.append(ut)
                        ci = 0
                        for c0_ in range(0, L, 512):
                            cl = min(512, L - c0_)
                            ps = ps_s.tile([128, 512], F32, tag="s")
                            nc.tensor.matmul(
                                ps[:, 0:cl],
                                lhsT=kt[:, qlo : qlo + 128],
                                rhs=qt[:, qlo + c0_ : qlo + c0_ + cl],
                                start=True,
                                stop=True,
                            )
                            if USE_DVE_EXP and h == 0 and ci % 2 == 1:
                                # first head: offload odd chunks to DVE exp
                                # (no PV to hide behind yet)
                                et = etp.tile([128, 512], F32, tag="et")
                                nc.vector._custom_dve(
                                    op_exp,
                                    out=et[:, 0:cl],
                                    in0=ps[:, 0:cl],
                                    s0=c2,
                                    s1=c1,
                                    imm2=c0,
                                )
                                nc.vector._custom_dve(
                                    op_pow4,
                                    out=ut[:, c0_ : c0_ + cl],
                                    in0=et[:, 0:cl],
                                )
                            else:
                                nc.scalar.activation(
                                    out=ut[:, c0_ : c0_ + cl],
                                    in_=ps[:, 0:cl],
                                    func=mybir.ActivationFunctionType.Exp,
                                    scale=float(SCALE),
                                )
                            ci += 1
                        # mask diagonal block: keep k <= q (partition <= free)
                        if GPSIMD_OPS:
                            nc.gpsimd.tensor_mul(ut[:, 0:128], ut[:, 0:128], trimask)
                        else:
                            nc.vector.tensor_mul(ut[:, 0:128], ut[:, 0:128], trimask)
                    if prev is not None:
                        pv_block(h - 1, kb, *prev)

                if prev is not None:
                    _, _, ohprev = prev
                    nc.sync.dma_start(out=o_r[h - 1], in_=ohprev)
                prev = cur
    _split_multi_waits(nc)
    return nc


_NC_CACHE = {}


def _get_nc(n_heads: int = HEADS_PER_CORE):
    if n_heads not in _NC_CACHE:
        _NC_CACHE[n_heads] = build_nc(n_heads)
    return _NC_CACHE[n_heads]


def make_in_maps(queries, keys, values):
    qf = np.ascontiguousarray(
        np.asarray(queries, dtype=np.float32).reshape(B * H, S, D)
    )
    kf = np.ascontiguousarray(np.asarray(keys, dtype=np.float32).reshape(B * H, S, D))
    vf = np.ascontiguousarray(
        np.asarray(values, dtype=np.float32).reshape(B * H, S, D)
    )
    n = HEADS_PER_CORE
    return [
        {
            "queries": qf[i * n : (i + 1) * n],
            "keys": kf[i * n : (i + 1) * n],
            "values": vf[i * n : (i + 1) * n],
        }
        for i in range(N_CORES)
    ]


def kernel(keys, queries, values, head_dim=None, **_ignored):
    nc = _get_nc()
    in_maps = make_in_maps(queries, keys, values)
    res = run_bass_kernel_spmd(nc, in_maps, core_ids=list(range(N_CORES)))
    out = np.concatenate([res.results[i]["out"] for i in range(N_CORES)], axis=0)
    return out.reshape(B, H, S, D).astype(np.float32)
